# revision 9
# baseline (speedup 1.0000x reference)
"""Block-causal GQA attention on 8 trn2 NeuronCores.

Sharding: core = b*4 + g  (b in {0,1} batch, g in {0..3} kv-head group).
Each core computes, for its batch b and kv group g (4 q-heads, 1 kv head):
    partial_out = softmax_blockcausal(rope(x@Wq_g) @ rope(x@Wk_g)^T) @ (x@Wv_g) @ Wo_g
Host sums the 4 group partials per batch.

Device layouts (all transposed-friendly, zero on-device transposes except a
cheap bf16 DMA-xbar transpose of V):
  inputs: xT [C,T] bf16 (host pre-transposed), wq [C,512], wk/wv [C,128],
          wo [512,C] bf16, cosT/sinT [128,T] f32 (sign of rotate-half folded
          into sinT).
  Q^T/K^T computed as Wq_chunk.T @ xT -> [d, T]; RoPE applied on DVE during
  PSUM eviction.  V computed as V^T then DMA-transposed.
  Attention per head in S^T layout: S^T[tk,tq] = K^T.T @ Q^T, exp on ACT
  (scale=1/sqrt(128), no max subtraction -- scores are O(1) for this data),
  Y^T[d,tq] += V.T @ P^T (V stationary), Z[1,tq] += ones.T @ P^T.
  Y^T normalized by 1/Z (DMA-broadcast) during eviction.
  O[t,n] = sum_h Y_h^T.T @ Wo_h accumulated in PSUM over heads.
"""
import os
import sys
import numpy as np

for _p in ("/opt/trn_rl_repo",):
    if _p not in sys.path and os.path.isdir(_p):
        sys.path.insert(0, _p)

import ml_dtypes

BF16 = ml_dtypes.bfloat16

B = 2
T = 2048
C = 2048
HD = 128
NHL = 4           # q heads per core
NT = T // 128     # 16 query/key tiles
NCH = C // 128    # 16 contraction chunks
SCALE = 1.0 / float(np.sqrt(np.float32(HD)))

_CACHE = {}


def _build_nc():
    import concourse.bass as bass
    import concourse.mybir as mybir
    import concourse.tile as tile
    from concourse import bacc

    dt = mybir.dt
    f32 = dt.float32
    bf = dt.bfloat16
    Exp = mybir.ActivationFunctionType.Exp

    nc = bacc.Bacc(None, target_bir_lowering=False)

    xT = nc.declare_dram_parameter("xT", [C, T], bf, isOutput=False)
    wq = nc.declare_dram_parameter("wq", [C, NHL * HD], bf, isOutput=False)
    wk = nc.declare_dram_parameter("wk", [C, HD], bf, isOutput=False)
    wv = nc.declare_dram_parameter("wv", [C, HD], bf, isOutput=False)
    wo = nc.declare_dram_parameter("wo", [NHL * HD, C], bf, isOutput=False)
    cosT = nc.declare_dram_parameter("cosT", [HD, T], f32, isOutput=False)
    sinT = nc.declare_dram_parameter("sinT", [HD, T], f32, isOutput=False)
    o = nc.declare_dram_parameter("o_part", [T, C], f32, isOutput=True)
    # DRAM bounce rows for the 1/Z partition-broadcast (one per (head, half))
    zscr = nc.dram_tensor("zscr", [NHL * 2, T // 2], f32)

    with tile.TileContext(nc) as tc:
        with tc.tile_pool(name="consts", bufs=1) as consts:
            # ---- static SBUF loads ----
            wk_sb = consts.tile([128, NCH, HD], bf, name="wk_sb")
            nc.sync.dma_start(wk_sb, wk.rearrange("(n p) m -> p n m", p=128))
            wv_sb = consts.tile([128, NCH, HD], bf, name="wv_sb")
            nc.sync.dma_start(wv_sb, wv.rearrange("(n p) m -> p n m", p=128))
            cos_sb = consts.tile([128, T], f32, name="cos_sb")
            nc.sync.dma_start(cos_sb, cosT[:, :])
            sin_sb = consts.tile([128, T], f32, name="sin_sb")
            nc.sync.dma_start(sin_sb, sinT[:, :])

            xt_r = xT.rearrange("(n p) t -> n p t", p=128)
            xt_sb = []
            for cch in range(NCH):
                xt_c = consts.tile([128, T], bf, name=f"xt{cch}")
                nc.sync.dma_start(xt_c, xt_r[cch])
                xt_sb.append(xt_c)

            wq_sb = consts.tile([128, NCH, NHL * HD], bf, name="wq_sb")
            nc.sync.dma_start(wq_sb, wq.rearrange("(n p) m -> p n m", p=128))
            wo_sb = consts.tile([128, NHL, C], bf, name="wo_sb")
            nc.sync.dma_start(wo_sb, wo.rearrange("(h p) m -> p h m", p=128))

            ones_sb = consts.tile([128, 1], bf, name="ones_sb")
            nc.vector.memset(ones_sb, 1.0)

            # warm the ACT exp table set during phase 1
            dumm = consts.tile([1, 8], f32, name="dumm")
            nc.vector.memset(dumm, 0.0)
            nc.scalar.activation(dumm, dumm, Exp)

            # persistent activations
            kt_sb = consts.tile([128, T], bf, name="kt_sb")
            vt_sb = consts.tile([128, T], bf, name="vt_sb")
            v_sb = consts.tile([128, NT, HD], bf, name="v_sb")
            qt_sb = [consts.tile([128, T], bf, name=f"qt{h}") for h in range(NHL)]
            yt_sb = [consts.tile([128, T], bf, name=f"yt{h}") for h in range(NHL)]

            # ================= phase 1: projections + RoPE =================
            with tc.tile_pool(name="proj", bufs=1) as proj:

                def rope_evict(ps, jsl, dst):
                    # dst[:, jsl] = ps * cos + rot_half(ps) * sin   (bf16 out)
                    t1 = proj.tile([128, 512], bf, tag="t1", bufs=3)
                    t2 = proj.tile([128, 512], bf, tag="t2", bufs=3)
                    nc.vector.tensor_mul(t1, ps, cos_sb[:, jsl])
                    nc.vector.tensor_mul(t2[0:64], ps[64:128], sin_sb[0:64, jsl])
                    nc.vector.tensor_mul(t2[64:128], ps[0:64], sin_sb[64:128, jsl])
                    nc.vector.tensor_add(dst[:, jsl], t1, t2)

                with tc.tile_pool(name="proj_psum", bufs=4, space="PSUM") as pp:
                    # K^T (+RoPE)
                    for j in range(T // 512):
                        jsl = slice(512 * j, 512 * (j + 1))
                        ps = pp.tile([128, 512], f32, tag="ps", bufs=4)
                        for cch in range(NCH):
                            nc.tensor.matmul(
                                ps, wk_sb[:, cch, :], xt_sb[cch][:, jsl],
                                start=(cch == 0), stop=(cch == NCH - 1),
                            )
                        rope_evict(ps, jsl, kt_sb)
                    # V^T (plain evict), then DMA-transpose to V
                    for j in range(T // 512):
                        jsl = slice(512 * j, 512 * (j + 1))
                        ps = pp.tile([128, 512], f32, tag="ps", bufs=4)
                        for cch in range(NCH):
                            nc.tensor.matmul(
                                ps, wv_sb[:, cch, :], xt_sb[cch][:, jsl],
                                start=(cch == 0), stop=(cch == NCH - 1),
                            )
                        nc.vector.tensor_copy(vt_sb[:, jsl], ps)
                    for i in range(NT):
                        nc.sync.dma_start_transpose(
                            v_sb[:, i, :], vt_sb[:, 128 * i:128 * (i + 1)]
                        )
                    # Q^T per head (+RoPE)
                    for h in range(NHL):
                        hsl = slice(HD * h, HD * (h + 1))
                        for j in range(T // 512):
                            jsl = slice(512 * j, 512 * (j + 1))
                            ps = pp.tile([128, 512], f32, tag="ps", bufs=4)
                            for cch in range(NCH):
                                nc.tensor.matmul(
                                    ps, wq_sb[:, cch, hsl], xt_sb[cch][:, jsl],
                                    start=(cch == 0), stop=(cch == NCH - 1),
                                )
                            rope_evict(ps, jsl, qt_sb[h])

            # ================= phase 2: attention per head =================
            HW = T // 2  # 1024-wide tq halves
            with tc.tile_pool(name="attn", bufs=1) as ap, \
                 tc.tile_pool(name="attn_psum", bufs=1, space="PSUM") as apsum:
                for h in range(NHL):
                    for half in range(2):
                        tq0 = HW * half
                        last_tk = (tq0 + HW) // 128 - 1
                        ps_y = apsum.tile([128, HW], f32, tag="y", bufs=1)
                        ps_z = apsum.tile([1, HW], f32, tag="z", bufs=1)

                        def yz_mms(tk, lo, p_t):
                            chunks = ([(lo, 512), (512, HW)] if lo < 512
                                      else [(lo, HW)])
                            st = (tk == 0)
                            sp = (tk == last_tk)
                            for (a, bnd) in chunks:
                                nc.tensor.matmul(
                                    ps_y[:, a:bnd], v_sb[:, tk, :],
                                    p_t[:, a:bnd], start=st, stop=sp,
                                )
                            for (a, bnd) in chunks:
                                nc.tensor.matmul(
                                    ps_z[:, a:bnd], ones_sb,
                                    p_t[:, a:bnd], start=st, stop=sp,
                                )

                        pend = None
                        for tk in range(last_tk + 1):
                            lo = max(0, 128 * tk - tq0)
                            ps_s = apsum.tile([128, HW], f32, tag="s", bufs=2)
                            chunks = ([(lo, 512), (512, HW)] if lo < 512
                                      else [(lo, HW)])
                            for (a, bnd) in chunks:
                                nc.tensor.matmul(
                                    ps_s[:, a:bnd],
                                    kt_sb[:, 128 * tk:128 * (tk + 1)],
                                    qt_sb[h][:, tq0 + a:tq0 + bnd],
                                    start=True, stop=True,
                                )
                            p_t = ap.tile([128, HW], bf, tag="p", bufs=3)
                            nc.scalar.activation(
                                p_t[:, lo:HW], ps_s[:, lo:HW], Exp, scale=SCALE
                            )
                            if pend is not None:
                                yz_mms(*pend)
                            pend = (tk, lo, p_t)
                        yz_mms(*pend)

                        # 1/Z, partition-broadcast via DRAM bounce, evict Y^T
                        rz = ap.tile([1, HW], f32, tag="rz", bufs=2)
                        nc.vector.reciprocal(rz, ps_z)
                        zrow = zscr[2 * h + half]
                        nc.sync.dma_start(zrow, rz)
                        rzb = ap.tile([128, HW], f32, tag="rzb", bufs=2)
                        bcast = bass.AP(
                            tensor=zrow.tensor, offset=zrow.offset,
                            ap=[[0, 128]] + list(zrow.ap),
                        )
                        nc.sync.dma_start(rzb, bcast)
                        nc.vector.tensor_mul(
                            yt_sb[h][:, tq0:tq0 + HW], ps_y, rzb
                        )

            # ================= phase 3: output projection =================
            with tc.tile_pool(name="oproj", bufs=1) as op, \
                 tc.tile_pool(name="oproj_psum", bufs=1, space="PSUM") as opsum:
                for ti in range(NT):
                    tsl = slice(128 * ti, 128 * (ti + 1))
                    for n in range(C // 512):
                        nsl = slice(512 * n, 512 * (n + 1))
                        ps_o = opsum.tile([128, 512], f32, tag="o", bufs=4)
                        for h in range(NHL):
                            nc.tensor.matmul(
                                ps_o, yt_sb[h][:, tsl], wo_sb[:, h, nsl],
                                start=(h == 0), stop=(h == NHL - 1),
                            )
                        ob = op.tile([128, 512], f32, tag="ob", bufs=4)
                        nc.vector.tensor_copy(ob, ps_o)
                        nc.sync.dma_start(o[tsl, nsl], ob)

    nc.finalize()
    return nc


def _tables():
    freqs = 1.0 / (10000.0 ** (np.arange(0, HD, 2, dtype=np.float32) / HD))
    t = np.arange(T, dtype=np.float32)
    emb = np.outer(t, freqs)                 # [T, 64]
    cos_t = np.cos(emb).T.astype(np.float32)  # [64, T]
    sin_t = np.sin(emb).T.astype(np.float32)
    cosT = np.ascontiguousarray(np.concatenate([cos_t, cos_t], 0))
    sinT = np.ascontiguousarray(np.concatenate([-sin_t, sin_t], 0))
    return cosT, sinT


def _get_nc():
    if "nc" not in _CACHE:
        _CACHE["nc"] = _build_nc()
    return _CACHE["nc"]


def kernel(x, Wq, Wk, Wv, Wo, _trace=False):
    from concourse.bass_utils import run_bass_kernel_spmd

    x = np.asarray(x, dtype=np.float32)
    cosT, sinT = _tables()
    in_maps = []
    for core in range(8):
        b, g = divmod(core, 4)
        in_maps.append({
            "xT": np.ascontiguousarray(x[b].T).astype(BF16),
            "wq": np.ascontiguousarray(Wq[:, 512 * g:512 * (g + 1)]).astype(BF16),
            "wk": np.ascontiguousarray(Wk[:, 128 * g:128 * (g + 1)]).astype(BF16),
            "wv": np.ascontiguousarray(Wv[:, 128 * g:128 * (g + 1)]).astype(BF16),
            "wo": np.ascontiguousarray(Wo[512 * g:512 * (g + 1), :]).astype(BF16),
            "cosT": cosT,
            "sinT": sinT,
        })

    nc = _get_nc()
    res = run_bass_kernel_spmd(nc, in_maps, list(range(8)), trace=_trace)
    parts = [res.results[c]["o_part"] for c in range(8)]
    out = np.empty((B, T, C), dtype=np.float32)
    for b in range(B):
        out[b] = parts[4 * b] + parts[4 * b + 1] + parts[4 * b + 2] + parts[4 * b + 3]
    if _trace:
        return out, res
    return out


# revision 15
# speedup vs baseline: 1.3388x; 1.3388x over previous
"""Block-causal GQA attention on 8 trn2 NeuronCores.

Sharding: core = b*4 + g  (b in {0,1} batch, g in {0..3} kv-head group).
Each core computes, for its batch b and kv group g (4 q-heads, 1 kv head):
    partial_out = softmax_blockcausal(rope(x@Wq_g) @ rope(x@Wk_g)^T) @ (x@Wv_g) @ Wo_g
Host sums the 4 group partials per batch.

Device design (bf16 matmuls, f32 PSUM):
  - Host passes x^T, so Q^T/K^T/V^T come out of projections with d on
    partitions and no on-device transposes; RoPE (sign folded into the sin
    table) happens on DVE during PSUM eviction.  V^T is DMA-xbar-transposed
    into V_aug = [V | ones].
  - Projections run c-chunk-outer in PSUM waves (K+V, Q0+Q1, Q2+Q3) so PE
    work starts as soon as the first x^T chunk lands.
  - Attention per (head, 1024-wide tq half): S^T[tk,tq] = K^T.T @ Q^T,
    exp on ACT (scale=1/sqrt(128); scores are O(1) so no max subtraction),
    then per 128-wide tq tile: [Y|Z][tq,129] += P^T_tile.T @ V_aug
    (P^T stationary, fused softmax denominator in column 128).
    Normalize: rz = 1/Z [tq,1], Y *= rz via per-partition tensor_scalar,
    DMA-transpose Y tile into Y^T[d, tq].
  - O[t,n] = sum_h Y_h^T.T @ Wo_h accumulated in PSUM over heads;
    evictions alternate DVE/ACT.
"""
import os
import sys
import numpy as np

for _p in ("/opt/trn_rl_repo",):
    if _p not in sys.path and os.path.isdir(_p):
        sys.path.insert(0, _p)

import ml_dtypes

BF16 = ml_dtypes.bfloat16

B = 2
T = 2048
C = 2048
HD = 128
NHL = 4           # q heads per core
NT = T // 128     # 16 query/key tiles
NCH = C // 128    # 16 contraction chunks
HW = T // 2       # tq half width
SCALE = 1.0 / float(np.sqrt(np.float32(HD)))

_CACHE = {}


def _build_nc():
    import concourse.bass as bass
    import concourse.mybir as mybir
    import concourse.tile as tile
    from concourse import bacc

    dt = mybir.dt
    f32 = dt.float32
    bf = dt.bfloat16
    Exp = mybir.ActivationFunctionType.Exp

    nc = bacc.Bacc(None, target_bir_lowering=False)

    xT = nc.declare_dram_parameter("xT", [C, T], bf, isOutput=False)
    wq = nc.declare_dram_parameter("wq", [C, NHL * HD], bf, isOutput=False)
    wk = nc.declare_dram_parameter("wk", [C, HD], bf, isOutput=False)
    wv = nc.declare_dram_parameter("wv", [C, HD], bf, isOutput=False)
    wo = nc.declare_dram_parameter("wo", [NHL * HD, C], bf, isOutput=False)
    cosT = nc.declare_dram_parameter("cosT", [HD, T], f32, isOutput=False)
    sinT = nc.declare_dram_parameter("sinT", [HD, T], f32, isOutput=False)
    o = nc.declare_dram_parameter("o_part", [T, C], f32, isOutput=True)

    with tile.TileContext(nc) as tc:
        with tc.tile_pool(name="consts", bufs=1) as consts:
            # ---- static SBUF loads (order = DMA priority) ----
            wk_sb = consts.tile([128, NCH, HD], bf, name="wk_sb")
            nc.sync.dma_start(wk_sb, wk.rearrange("(n p) m -> p n m", p=128))
            wv_sb = consts.tile([128, NCH, HD], bf, name="wv_sb")
            nc.sync.dma_start(wv_sb, wv.rearrange("(n p) m -> p n m", p=128))

            cos_sb = consts.tile([128, T], f32, name="cos_sb")
            sin_sb = consts.tile([128, T], f32, name="sin_sb")
            wq_sb = consts.tile([128, NCH, NHL * HD], bf, name="wq_sb")
            wo_sb = consts.tile([128, NHL, C], bf, name="wo_sb")

            # V_aug = [V | ones]: col 128 preset to 1, cols 0:128 filled by
            # DMA-transpose from V^T after the V projection.  Rows are 256
            # wide so each tile's dst offset stays 512B-aligned — the DMA
            # xbar transpose corrupts data at unaligned dst offsets.
            vaug_sb = consts.tile([128, NT, 2 * HD], bf, name="vaug_sb")
            nc.vector.memset(vaug_sb[:, :, HD:HD + 1], 1.0)

            # warm the ACT exp table set during phase 1
            dumm = consts.tile([1, 8], f32, name="dumm")
            nc.vector.memset(dumm, 0.0)
            nc.scalar.activation(dumm, dumm, Exp)

            # persistent activations
            kt_sb = consts.tile([128, T], bf, name="kt_sb")
            vt_sb = consts.tile([128, T], bf, name="vt_sb")
            qt_sb = [consts.tile([128, T], bf, name=f"qt{h}") for h in range(NHL)]
            yt_sb = [consts.tile([128, T], bf, name=f"yt{h}") for h in range(NHL)]

            # ============ phase 1: projections (c-outer waves) ============
            with tc.tile_pool(name="xtp", bufs=1) as xtp, \
                 tc.tile_pool(name="proj", bufs=1) as proj, \
                 tc.tile_pool(name="proj_psum", bufs=2, space="PSUM") as pp:

                xt_r = xT.rearrange("(n p) t -> n p t", p=128)
                xt_sb = []
                for cch in range(NCH):
                    xt_c = xtp.tile([128, T], bf, name=f"xt{cch}")
                    nc.sync.dma_start(xt_c, xt_r[cch])
                    xt_sb.append(xt_c)
                    if cch == 10:
                        # wq arrives just before the Q waves need it
                        nc.sync.dma_start(
                            wq_sb, wq.rearrange("(n p) m -> p n m", p=128))
                nc.sync.dma_start(cos_sb, cosT[:, :])
                nc.sync.dma_start(sin_sb, sinT[:, :])
                nc.sync.dma_start(wo_sb, wo.rearrange("(h p) m -> p h m", p=128))

                def rope_evict(ps, jsl, dst):
                    # dst[:, jsl] = ps * cos + rot_half(ps) * sin  (bf16 out)
                    t1 = proj.tile([128, 512], bf, tag="t1", bufs=3)
                    t2 = proj.tile([128, 512], bf, tag="t2", bufs=3)
                    nc.vector.tensor_mul(t1, ps, cos_sb[:, jsl])
                    nc.vector.tensor_mul(t2[0:64], ps[64:128], sin_sb[0:64, jsl])
                    nc.vector.tensor_mul(t2[64:128], ps[0:64], sin_sb[64:128, jsl])
                    nc.vector.tensor_add(dst[:, jsl], t1, t2)

                # -- wave 1: K and V --
                ps_k = pp.tile([128, T], f32, tag="pw", bufs=2, name="ps_k")
                ps_v = pp.tile([128, T], f32, tag="pw", bufs=2, name="ps_v")
                for cch in range(NCH):
                    st, sp = (cch == 0), (cch == NCH - 1)
                    for j in range(T // 512):
                        jsl = slice(512 * j, 512 * (j + 1))
                        nc.tensor.matmul(ps_k[:, jsl], wk_sb[:, cch, :],
                                         xt_sb[cch][:, jsl], start=st, stop=sp)
                        nc.tensor.matmul(ps_v[:, jsl], wv_sb[:, cch, :],
                                         xt_sb[cch][:, jsl], start=st, stop=sp)
                for j in range(T // 512):
                    jsl = slice(512 * j, 512 * (j + 1))
                    rope_evict(ps_k[:, jsl], jsl, kt_sb)
                    nc.vector.tensor_copy(vt_sb[:, jsl], ps_v[:, jsl])
                for i in range(NT):
                    nc.sync.dma_start_transpose(
                        vaug_sb[:, i, 0:HD], vt_sb[:, 128 * i:128 * (i + 1)]
                    )

                # -- waves 2+3: Q head pairs --
                for h0 in (0, 2):
                    ps_q = [pp.tile([128, T], f32, tag="pw", bufs=2,
                                    name=f"ps_q{h0 + d}") for d in (0, 1)]
                    for cch in range(NCH):
                        st, sp = (cch == 0), (cch == NCH - 1)
                        for d in (0, 1):
                            hsl = slice(HD * (h0 + d), HD * (h0 + d + 1))
                            for j in range(T // 512):
                                jsl = slice(512 * j, 512 * (j + 1))
                                nc.tensor.matmul(
                                    ps_q[d][:, jsl], wq_sb[:, cch, hsl],
                                    xt_sb[cch][:, jsl], start=st, stop=sp)
                    for d in (0, 1):
                        for j in range(T // 512):
                            jsl = slice(512 * j, 512 * (j + 1))
                            rope_evict(ps_q[d][:, jsl], jsl, qt_sb[h0 + d])

            # ============ phase 2: attention ============
            with tc.tile_pool(name="attn", bufs=1) as ap, \
                 tc.tile_pool(name="attn_psum", bufs=1, space="PSUM") as apsum:

                def s_block(h, half):
                    """S^T + exp for all tk tiles of this tq half."""
                    tq0 = HW * half
                    tiles = []
                    for tk in range((tq0 + HW) // 128):
                        lo = max(0, 128 * tk - tq0)
                        ps_s = apsum.tile([128, HW], f32, tag="s", bufs=2)
                        chunks = ([(lo, 512), (512, HW)] if lo < 512
                                  else [(lo, HW)])
                        for (a, bnd) in chunks:
                            nc.tensor.matmul(
                                ps_s[:, a:bnd],
                                kt_sb[:, 128 * tk:128 * (tk + 1)],
                                qt_sb[h][:, tq0 + a:tq0 + bnd],
                                start=True, stop=True)
                        p_t = ap.tile([128, HW], bf, tag="p", bufs=26)
                        nc.scalar.activation(p_t[:, lo:HW], ps_s[:, lo:HW],
                                             Exp, scale=SCALE)
                        tiles.append(p_t)
                    return tiles

                def y_block(h, half, tiles):
                    """[Y|Z] accumulation + normalize + transpose-out."""
                    for il in range(HW // 128):
                        gi = (HW // 128) * half + il
                        ps_yz = apsum.tile([128, 132], f32, tag="yz", bufs=4)
                        for tk in range(gi + 1):
                            nc.tensor.matmul(
                                ps_yz[:, 0:HD + 1],
                                tiles[tk][:, 128 * il:128 * (il + 1)],
                                vaug_sb[:, tk, 0:HD + 1],
                                start=(tk == 0), stop=(tk == gi))
                        rz = ap.tile([128, 1], f32, tag="rz", bufs=4)
                        nc.vector.reciprocal(rz, ps_yz[:, HD:HD + 1])
                        ysb = ap.tile([128, HD], bf, tag="ysb", bufs=4)
                        nc.vector.tensor_scalar_mul(ysb, ps_yz[:, 0:HD], rz)
                        nc.sync.dma_start_transpose(
                            yt_sb[h][:, 128 * gi:128 * (gi + 1)], ysb)

                prev = None
                for h in range(NHL):
                    for half in range(2):
                        cur = (h, half, s_block(h, half))
                        if prev is not None:
                            y_block(*prev)
                        prev = cur
                y_block(*prev)

            # ============ phase 3: output projection ============
            with tc.tile_pool(name="oproj", bufs=1) as op, \
                 tc.tile_pool(name="oproj_psum", bufs=1, space="PSUM") as opsum:
                for ti in range(NT):
                    tsl = slice(128 * ti, 128 * (ti + 1))
                    for n in range(C // 512):
                        nsl = slice(512 * n, 512 * (n + 1))
                        ps_o = opsum.tile([128, 512], f32, tag="o", bufs=4)
                        for h in range(NHL):
                            nc.tensor.matmul(
                                ps_o, yt_sb[h][:, tsl], wo_sb[:, h, nsl],
                                start=(h == 0), stop=(h == NHL - 1))
                        ob = op.tile([128, 512], f32, tag="ob", bufs=6)
                        if (ti * 4 + n) % 2 == 0:
                            nc.vector.tensor_copy(ob, ps_o)
                        else:
                            nc.scalar.copy(ob, ps_o)
                        nc.sync.dma_start(o[tsl, nsl], ob)

    nc.finalize()
    return nc


def _tables():
    freqs = 1.0 / (10000.0 ** (np.arange(0, HD, 2, dtype=np.float32) / HD))
    t = np.arange(T, dtype=np.float32)
    emb = np.outer(t, freqs)                  # [T, 64]
    cos_t = np.cos(emb).T.astype(np.float32)  # [64, T]
    sin_t = np.sin(emb).T.astype(np.float32)
    cosT = np.ascontiguousarray(np.concatenate([cos_t, cos_t], 0))
    sinT = np.ascontiguousarray(np.concatenate([-sin_t, sin_t], 0))
    return cosT, sinT


def _get_nc():
    if "nc" not in _CACHE:
        _CACHE["nc"] = _build_nc()
    return _CACHE["nc"]


def kernel(x, Wq, Wk, Wv, Wo, _trace=False):
    from concourse.bass_utils import run_bass_kernel_spmd

    x = np.asarray(x, dtype=np.float32)
    cosT, sinT = _tables()
    in_maps = []
    for core in range(8):
        b, g = divmod(core, 4)
        in_maps.append({
            "xT": np.ascontiguousarray(x[b].T).astype(BF16),
            "wq": np.ascontiguousarray(Wq[:, 512 * g:512 * (g + 1)]).astype(BF16),
            "wk": np.ascontiguousarray(Wk[:, 128 * g:128 * (g + 1)]).astype(BF16),
            "wv": np.ascontiguousarray(Wv[:, 128 * g:128 * (g + 1)]).astype(BF16),
            "wo": np.ascontiguousarray(Wo[512 * g:512 * (g + 1), :]).astype(BF16),
            "cosT": cosT,
            "sinT": sinT,
        })

    nc = _get_nc()
    res = run_bass_kernel_spmd(nc, in_maps, list(range(8)), trace=_trace)
    parts = [res.results[c]["o_part"] for c in range(8)]
    out = np.empty((B, T, C), dtype=np.float32)
    for b in range(B):
        out[b] = parts[4 * b] + parts[4 * b + 1] + parts[4 * b + 2] + parts[4 * b + 3]
    if _trace:
        return out, res
    return out


# revision 21
# speedup vs baseline: 1.4941x; 1.1160x over previous
"""Block-causal GQA attention on 8 trn2 NeuronCores.

Sharding: core = b*4 + g  (b in {0,1} batch, g in {0..3} kv-head group).
Each core computes, for its batch b and kv group g (4 q-heads, 1 kv head):
    partial_out = softmax_blockcausal(rope(x@Wq_g) @ rope(x@Wk_g)^T) @ (x@Wv_g) @ Wo_g
Host sums the 4 group partials per batch.

Device design (bf16 matmuls, f32 PSUM):
  - Host passes x^T, so Q^T/K^T/V^T come out of projections with d on
    partitions and no on-device transposes; RoPE (sign folded into the sin
    table) happens on DVE during PSUM eviction.  V^T is DMA-xbar-transposed
    into V_aug = [V | ones].
  - Projections run c-chunk-outer in PSUM waves (K+V, Q0+Q1, Q2+Q3) so PE
    work starts as soon as the first x^T chunk lands.
  - Attention per (head, 1024-wide tq half): S^T[tk,tq] = K^T.T @ Q^T,
    exp on ACT (scale=1/sqrt(128); scores are O(1) so no max subtraction),
    then per 128-wide tq tile: [Y|Z][tq,129] += P^T_tile.T @ V_aug
    (P^T stationary, fused softmax denominator in column 128).
    Normalize: rz = 1/Z [tq,1], Y *= rz via per-partition tensor_scalar,
    DMA-transpose Y tile into Y^T[d, tq].
  - O[t,n] = sum_h Y_h^T.T @ Wo_h accumulated in PSUM over heads;
    evictions alternate DVE/ACT.
"""
import os
import sys
import numpy as np

for _p in ("/opt/trn_rl_repo",):
    if _p not in sys.path and os.path.isdir(_p):
        sys.path.insert(0, _p)

import ml_dtypes

BF16 = ml_dtypes.bfloat16

B = 2
T = 2048
C = 2048
HD = 128
NHL = 4           # q heads per core
NT = T // 128     # 16 query/key tiles
NCH = C // 128    # 16 contraction chunks
HW = T // 2       # tq half width
SCALE = 1.0 / float(np.sqrt(np.float32(HD)))

_CACHE = {}


def _build_nc():
    import concourse.bass as bass
    import concourse.mybir as mybir
    import concourse.tile as tile
    from concourse import bacc

    dt = mybir.dt
    f32 = dt.float32
    bf = dt.bfloat16
    Exp = mybir.ActivationFunctionType.Exp

    nc = bacc.Bacc(None, target_bir_lowering=False)

    xT = nc.declare_dram_parameter("xT", [C, T], bf, isOutput=False)
    wq = nc.declare_dram_parameter("wq", [C, NHL * HD], bf, isOutput=False)
    wk = nc.declare_dram_parameter("wk", [C, HD], bf, isOutput=False)
    wv = nc.declare_dram_parameter("wv", [C, HD], bf, isOutput=False)
    wo = nc.declare_dram_parameter("wo", [NHL * HD, C], bf, isOutput=False)
    cosT = nc.declare_dram_parameter("cosT", [HD, T], bf, isOutput=False)
    sinT = nc.declare_dram_parameter("sinT", [HD, T], bf, isOutput=False)
    o = nc.declare_dram_parameter("o_part", [T, C], f32, isOutput=True)

    with tile.TileContext(nc) as tc:
        with tc.tile_pool(name="consts", bufs=1) as consts:
            # ---- static SBUF loads (order = DMA priority) ----
            wk_sb = consts.tile([128, NCH, HD], bf, name="wk_sb")
            nc.sync.dma_start(wk_sb, wk.rearrange("(n p) m -> p n m", p=128))
            wv_sb = consts.tile([128, NCH, HD], bf, name="wv_sb")
            nc.sync.dma_start(wv_sb, wv.rearrange("(n p) m -> p n m", p=128))

            cos_sb = consts.tile([128, T], bf, name="cos_sb")
            sin_sb = consts.tile([128, T], bf, name="sin_sb")
            wq_sb = consts.tile([128, NCH, NHL * HD], bf, name="wq_sb")
            wo_sb = consts.tile([128, NHL, C], bf, name="wo_sb")

            # V_aug = [V | ones]: col 128 preset to 1, cols 0:128 filled by
            # DMA-transpose from V^T after the V projection.  Rows are 256
            # wide so each tile's dst offset stays 512B-aligned — the DMA
            # xbar transpose corrupts data at unaligned dst offsets.
            vaug_sb = consts.tile([128, NT, 2 * HD], bf, name="vaug_sb")
            nc.vector.memset(vaug_sb[:, :, HD:HD + 1], 1.0)

            # warm the ACT exp table set during phase 1
            dumm = consts.tile([1, 8], f32, name="dumm")
            nc.vector.memset(dumm, 0.0)
            nc.scalar.activation(dumm, dumm, Exp)

            # persistent activations
            kt_sb = consts.tile([128, T], bf, name="kt_sb")
            vt_sb = consts.tile([128, T], bf, name="vt_sb")
            qt_sb = [consts.tile([128, T], bf, name=f"qt{h}") for h in range(NHL)]
            yt_sb = [consts.tile([128, T], bf, name=f"yt{h}") for h in range(NHL)]

            # ============ phase 1: projections (c-outer waves) ============
            with tc.tile_pool(name="xtp", bufs=1) as xtp, \
                 tc.tile_pool(name="proj", bufs=1) as proj, \
                 tc.tile_pool(name="proj_psum", bufs=2, space="PSUM") as pp:

                xt_r = xT.rearrange("(n p) t -> n p t", p=128)
                xt_sb = []
                for cch in range(NCH):
                    xt_c = xtp.tile([128, T], bf, name=f"xt{cch}")
                    nc.sync.dma_start(xt_c, xt_r[cch])
                    xt_sb.append(xt_c)
                    if cch == 10:
                        # wq arrives just before the Q waves need it
                        nc.sync.dma_start(
                            wq_sb, wq.rearrange("(n p) m -> p n m", p=128))
                nc.sync.dma_start(cos_sb, cosT[:, :])
                nc.sync.dma_start(sin_sb, sinT[:, :])
                nc.sync.dma_start(wo_sb, wo.rearrange("(h p) m -> p h m", p=128))

                def rope_evict(ps, jsl, dst):
                    # dst[:, jsl] = ps * cos + rot_half(ps) * sin  (bf16).
                    # ACT does the PSUM eviction; DVE runs at bf16 2x.
                    t0 = proj.tile([128, 512], bf, tag="t0", bufs=4)
                    t1 = proj.tile([128, 512], bf, tag="t1", bufs=4)
                    t2 = proj.tile([128, 512], bf, tag="t2", bufs=4)
                    # sin table halves are pre-swapped on host so each mul
                    # reads both SBUF inputs at the same base partition
                    # (walrus requires equal SBUF base partitions).
                    nc.scalar.copy(t0, ps)
                    nc.vector.tensor_mul(t1, t0, cos_sb[:, jsl])
                    nc.vector.tensor_mul(t2[0:64], t0[64:128], sin_sb[64:128, jsl])
                    nc.vector.tensor_mul(t2[64:128], t0[0:64], sin_sb[0:64, jsl])
                    nc.vector.tensor_add(dst[:, jsl], t1, t2)

                # -- wave 1: K and V (c-outer so PE starts with first chunk) --
                ps_k = [pp.tile([128, 512], f32, tag="pj", bufs=8,
                                name=f"ps_k{j}") for j in range(4)]
                ps_v = [pp.tile([128, 512], f32, tag="pj", bufs=8,
                                name=f"ps_v{j}") for j in range(4)]
                for cch in range(NCH):
                    st, sp = (cch == 0), (cch == NCH - 1)
                    for j in range(T // 512):
                        jsl = slice(512 * j, 512 * (j + 1))
                        nc.tensor.matmul(ps_k[j], wk_sb[:, cch, :],
                                         xt_sb[cch][:, jsl], start=st, stop=sp)
                        nc.tensor.matmul(ps_v[j], wv_sb[:, cch, :],
                                         xt_sb[cch][:, jsl], start=st, stop=sp)
                for j in range(T // 512):
                    jsl = slice(512 * j, 512 * (j + 1))
                    rope_evict(ps_k[j], jsl, kt_sb)
                    nc.scalar.copy(vt_sb[:, jsl], ps_v[j])
                for i in range(NT):
                    nc.sync.dma_start_transpose(
                        vaug_sb[:, i, 0:HD], vt_sb[:, 128 * i:128 * (i + 1)]
                    )

                # -- Q: (h, j)-sequential, c-inner; evictions pipeline via
                #    the 8-slot psum rotation --
                for h in range(NHL):
                    hsl = slice(HD * h, HD * (h + 1))
                    for j in range(T // 512):
                        jsl = slice(512 * j, 512 * (j + 1))
                        ps_q = pp.tile([128, 512], f32, tag="pj", bufs=8,
                                       name=f"ps_q{h}_{j}")
                        for cch in range(NCH):
                            nc.tensor.matmul(
                                ps_q, wq_sb[:, cch, hsl], xt_sb[cch][:, jsl],
                                start=(cch == 0), stop=(cch == NCH - 1))
                        rope_evict(ps_q, jsl, qt_sb[h])

            # ============ phase 2: attention ============
            with tc.tile_pool(name="attn", bufs=1) as ap, \
                 tc.tile_pool(name="attn_psum", bufs=1, space="PSUM") as apsum:

                def s_block(h, half):
                    """S^T + exp for all tk tiles of this tq half."""
                    tq0 = HW * half
                    tiles = []
                    for tk in range((tq0 + HW) // 128):
                        lo = max(0, 128 * tk - tq0)
                        ps_s = apsum.tile([128, HW], f32, tag="s", bufs=2)
                        chunks = ([(lo, 512), (512, HW)] if lo < 512
                                  else [(lo, HW)])
                        for (a, bnd) in chunks:
                            nc.tensor.matmul(
                                ps_s[:, a:bnd],
                                kt_sb[:, 128 * tk:128 * (tk + 1)],
                                qt_sb[h][:, tq0 + a:tq0 + bnd],
                                start=True, stop=True)
                        p_t = ap.tile([128, HW], bf, tag="p", bufs=26)
                        nc.scalar.activation(p_t[:, lo:HW], ps_s[:, lo:HW],
                                             Exp, scale=SCALE)
                        tiles.append(p_t)
                    return tiles

                def y_block(h, half, tiles):
                    """[Y|Z] accumulation + normalize + transpose-out."""
                    for il in range(HW // 128):
                        gi = (HW // 128) * half + il
                        ps_yz = apsum.tile([128, 132], f32, tag="yz", bufs=4)
                        for tk in range(gi + 1):
                            nc.tensor.matmul(
                                ps_yz[:, 0:HD + 1],
                                tiles[tk][:, 128 * il:128 * (il + 1)],
                                vaug_sb[:, tk, 0:HD + 1],
                                start=(tk == 0), stop=(tk == gi))
                        rz = ap.tile([128, 1], f32, tag="rz", bufs=4)
                        nc.vector.reciprocal(rz, ps_yz[:, HD:HD + 1])
                        ysb = ap.tile([128, HD], bf, tag="ysb", bufs=4)
                        nc.vector.tensor_scalar_mul(ysb, ps_yz[:, 0:HD], rz)
                        nc.sync.dma_start_transpose(
                            yt_sb[h][:, 128 * gi:128 * (gi + 1)], ysb)

                prev = None
                for h in range(NHL):
                    for half in range(2):
                        cur = (h, half, s_block(h, half))
                        if prev is not None:
                            y_block(*prev)
                        prev = cur
                y_block(*prev)

            # ============ phase 3: output projection ============
            with tc.tile_pool(name="oproj", bufs=1) as op, \
                 tc.tile_pool(name="oproj_psum", bufs=1, space="PSUM") as opsum:
                for ti in range(NT):
                    tsl = slice(128 * ti, 128 * (ti + 1))
                    for n in range(C // 512):
                        nsl = slice(512 * n, 512 * (n + 1))
                        ps_o = opsum.tile([128, 512], f32, tag="o", bufs=4)
                        for h in range(NHL):
                            nc.tensor.matmul(
                                ps_o, yt_sb[h][:, tsl], wo_sb[:, h, nsl],
                                start=(h == 0), stop=(h == NHL - 1))
                        ob = op.tile([128, 512], f32, tag="ob", bufs=6)
                        if (ti * 4 + n) % 2 == 0:
                            nc.vector.tensor_copy(ob, ps_o)
                        else:
                            nc.scalar.copy(ob, ps_o)
                        nc.sync.dma_start(o[tsl, nsl], ob)

    nc.finalize()
    return nc


def _tables():
    freqs = 1.0 / (10000.0 ** (np.arange(0, HD, 2, dtype=np.float32) / HD))
    t = np.arange(T, dtype=np.float32)
    emb = np.outer(t, freqs)                  # [T, 64]
    cos_t = np.cos(emb).T.astype(np.float32)  # [64, T]
    sin_t = np.sin(emb).T.astype(np.float32)
    cosT = np.ascontiguousarray(np.concatenate([cos_t, cos_t], 0)).astype(BF16)
    # halves swapped: row d holds the factor multiplying t0[(d+64)%128]
    # when writing t2[d ^ 64 half]; see rope_evict
    sinT = np.ascontiguousarray(np.concatenate([sin_t, -sin_t], 0)).astype(BF16)
    return cosT, sinT


def _get_nc():
    if "nc" not in _CACHE:
        _CACHE["nc"] = _build_nc()
    return _CACHE["nc"]


def kernel(x, Wq, Wk, Wv, Wo, _trace=False):
    from concourse.bass_utils import run_bass_kernel_spmd

    x = np.asarray(x, dtype=np.float32)
    cosT, sinT = _tables()
    in_maps = []
    for core in range(8):
        b, g = divmod(core, 4)
        in_maps.append({
            "xT": np.ascontiguousarray(x[b].T).astype(BF16),
            "wq": np.ascontiguousarray(Wq[:, 512 * g:512 * (g + 1)]).astype(BF16),
            "wk": np.ascontiguousarray(Wk[:, 128 * g:128 * (g + 1)]).astype(BF16),
            "wv": np.ascontiguousarray(Wv[:, 128 * g:128 * (g + 1)]).astype(BF16),
            "wo": np.ascontiguousarray(Wo[512 * g:512 * (g + 1), :]).astype(BF16),
            "cosT": cosT,
            "sinT": sinT,
        })

    nc = _get_nc()
    res = run_bass_kernel_spmd(nc, in_maps, list(range(8)), trace=_trace)
    parts = [res.results[c]["o_part"] for c in range(8)]
    out = np.empty((B, T, C), dtype=np.float32)
    for b in range(B):
        out[b] = parts[4 * b] + parts[4 * b + 1] + parts[4 * b + 2] + parts[4 * b + 3]
    if _trace:
        return out, res
    return out
